# revision 17
# baseline (speedup 1.0000x reference)
"""Multi-head attention on 8 TRN2 NeuronCores (Bass/Tile, SPMD, no collectives).

Problem: B=4, Sf=St=2048, DIM=768, H=12, Dh=64, f32 reference.

Sharding: (batch, Sf/2) -> 8 shards. Core c handles batch b=c//2, query rows
[(c%2)*1024 : +1024). K/V projections for a batch are recomputed by both cores
of the pair (cheaper than any collective).

Device dataflow is fully transposed so no on-chip transposes are needed:
  QT[h]  [64,1024]  = Wq_h^T @ xf^T     (lhsT=Wq cols, rhs=xf^T; bq folded in)
  KT[h]  [64,2048]  = Wk_h^T @ xt^T     (KT[1..5] computed DURING attention)
  V      [2048,768] = (xt^T)^T @ Wv
  S^T    [St,Sq]    per (head-pair, st-tile): two K=64 matmuls ROW-PACKED via
                    tile_position (0,0)/(64,0) into one [128,1024] PSUM tile
                    laid out [h0 sq-half | h1 sq-half] so both pair members
                    share one dependency and the pack never breaks
  P^T    = exp(S^T) * mask^T            (no max-subtract: scores are bounded,
                    masked lanes underflow to 0 exactly like the reference)
  ctx^T  [128,1024] = V^T @ P^T, two heads COL-PACKED via (0,0)/(0,64)
  Z      = 4x M=1 ones-matmuls col-packed at (0,32j), accumulated in PSUM
  1/Z    via DVE reciprocal_approx_fast + stride-0 broadcast DMA (DRAM bounce)
  out^T  [768,1024] = Wo^T @ (ctx^T * 1/Z) + bo'
Host transposes out^T back and stitches the 8 shards.

Exact bias folds (all exact math, not approximations):
  - bk: its score terms are constant along the softmax axis -> dropped
  - bv: softmax weights sum to 1 -> folded into output bias bo' = bv@Wo + bo
  - bq: folded (scaled) into the Q projection bias; SCALE folded into Wq
"""

import os
import numpy as np
import ml_dtypes

BF16 = ml_dtypes.bfloat16

B, SF, ST, DIM = 4, 2048, 2048, 768
NH, HD = 12, 64
SCALE = HD ** -0.5
NCORES = 8
ROWS = B * SF // NCORES      # 1024 query rows per core
HP = NH // 2                 # 6 head-pairs == 6 x 128-partition chunks of DIM
VW = DIM                     # 768: V width (no ones columns; Z via packed M=1 matmuls)
NST = ST // 128              # 16 st tiles

_CACHED_NC = None


def _build_nc():
    from concourse import bacc, tile, mybir
    import concourse.bass as bass

    dt = mybir.dt
    nc = bacc.Bacc("TRN2", target_bir_lowering=False, debug=False,
                   num_devices=NCORES)

    xfT = nc.dram_tensor("xfT", [DIM, ROWS], dt.bfloat16, kind="ExternalInput").ap()
    xtT = nc.dram_tensor("xtT", [DIM, ST], dt.bfloat16, kind="ExternalInput").ap()
    maskT = nc.dram_tensor("maskT", [ST, ROWS], dt.bfloat16, kind="ExternalInput").ap()
    wq = nc.dram_tensor("wq", [DIM, DIM], dt.bfloat16, kind="ExternalInput").ap()
    wk = nc.dram_tensor("wk", [DIM, DIM], dt.bfloat16, kind="ExternalInput").ap()
    wv = nc.dram_tensor("wv", [DIM, VW], dt.bfloat16, kind="ExternalInput").ap()
    wo = nc.dram_tensor("wo", [DIM, DIM], dt.bfloat16, kind="ExternalInput").ap()
    biases = nc.dram_tensor("biases", [128, 2 * HP], dt.float32, kind="ExternalInput").ap()
    out = nc.dram_tensor("out", [DIM, ROWS], dt.float32, kind="ExternalOutput").ap()
    rz_dram = nc.dram_tensor("rz_scratch", [NH, ROWS], dt.bfloat16).ap()

    EXP = mybir.ActivationFunctionType.Exp

    with tile.TileContext(nc) as tc:
        persist_cm = tc.tile_pool(name="persist", bufs=1)
        persist = persist_cm.__enter__()

        wo_sb = []
        for k in range(HP):
            t = persist.tile([128, DIM], dt.bfloat16, tag=f"wo{k}", name=f"wo{k}")
            nc.sync.dma_start(out=t, in_=wo[k * 128:(k + 1) * 128, :])
            wo_sb.append(t)
        bias_sb = persist.tile([128, 2 * HP], dt.float32, tag="biases", name="biases")
        nc.sync.dma_start(out=bias_sb, in_=biases)

        qt_sb = [persist.tile([128, ROWS], dt.bfloat16, tag=f"qt{i}", name=f"qt{i}") for i in range(HP)]
        kt_sb = [persist.tile([128, ST], dt.bfloat16, tag=f"kt{i}", name=f"kt{i}") for i in range(HP)]
        v_sb = [persist.tile([128, VW], dt.bfloat16, tag=f"v{i}", name=f"v{i}") for i in range(NST)]
        ctx_sb = [persist.tile([128, ROWS], dt.bfloat16, tag=f"ctx{i}", name=f"ctx{i}") for i in range(HP)]

        # ---------------- phase A: QT (all), KT[0], V upfront ----------------
        # xt + wk[1..5] live in projB (stays open: KT[1..5] is computed inside
        # the attention loop, one round per few steps, using 1 spare PSUM bank).
        projB = tc.tile_pool(name="projB", bufs=1)
        projB_cm = projB
        projB = projB.__enter__()
        xt_sb, wk_sb = [], []
        for k in range(HP):
            t = projB.tile([128, ST], dt.bfloat16, tag=f"xt{k}", name=f"xt{k}")
            nc.sync.dma_start(out=t, in_=xtT[k * 128:(k + 1) * 128, :])
            xt_sb.append(t)
            t = projB.tile([128, DIM], dt.bfloat16, tag=f"wk{k}", name=f"wk{k}")
            nc.sync.dma_start(out=t, in_=wk[k * 128:(k + 1) * 128, :])
            wk_sb.append(t)

        def kt_round(hp, n0, psum_pool):
            ps = psum_pool.tile([128, 512], dt.float32, tag="psA", name="psA")
            for k in range(HP):
                nc.tensor.matmul(
                    ps, wk_sb[k][:, hp * 128:(hp + 1) * 128],
                    xt_sb[k][:, n0:n0 + 512],
                    start=(k == 0), stop=(k == HP - 1))
            nc.vector.tensor_copy(out=kt_sb[hp][:, n0:n0 + 512], in_=ps)

        with tc.tile_pool(name="projIn", bufs=1) as projin, \
             tc.tile_pool(name="psA", bufs=4, space="PSUM") as psA:
            xf_sb, wq_sb, wv_sb = [], [], []
            for k in range(HP):
                t = projin.tile([128, ROWS], dt.bfloat16, tag=f"xf{k}", name=f"xf{k}")
                nc.sync.dma_start(out=t, in_=xfT[k * 128:(k + 1) * 128, :])
                xf_sb.append(t)
                t = projin.tile([128, DIM], dt.bfloat16, tag=f"wq{k}", name=f"wq{k}")
                nc.sync.dma_start(out=t, in_=wq[k * 128:(k + 1) * 128, :])
                wq_sb.append(t)
            for k in range(HP):
                t = projin.tile([128, VW], dt.bfloat16, tag=f"wv{k}", name=f"wv{k}")
                nc.sync.dma_start(out=t, in_=wv[k * 128:(k + 1) * 128, :])
                wv_sb.append(t)

            # QT: per head-pair hp, [128, ROWS] = sum_k wq[k][:,hp]^T @ xf[k]
            for hp in range(HP):
                for n0 in range(0, ROWS, 512):
                    ps = psA.tile([128, 512], dt.float32, tag="psA", name="psA")
                    for k in range(HP):
                        nc.tensor.matmul(
                            ps, wq_sb[k][:, hp * 128:(hp + 1) * 128],
                            xf_sb[k][:, n0:n0 + 512],
                            start=(k == 0), stop=(k == HP - 1))
                    nc.vector.tensor_scalar_add(
                        out=qt_sb[hp][:, n0:n0 + 512], in0=ps,
                        scalar1=bias_sb[:, hp:hp + 1])
            # KT[0] only (rest during attention)
            for n0 in range(0, ST, 512):
                kt_round(0, n0, psA)
            # V (+bias row): [128st, VW], lhsT=xt chunks
            for st in range(NST):
                c0 = st * 128
                for n0, nw in ((0, 512), (512, VW - 512)):
                    ps = psA.tile([128, 512], dt.float32, tag="psA", name="psA")
                    for k in range(HP):
                        nc.tensor.matmul(
                            ps[:, :nw], xt_sb[k][:, c0:c0 + 128],
                            wv_sb[k][:, n0:n0 + nw],
                            start=(k == 0), stop=(k == HP - 1))
                    nc.vector.tensor_copy(out=v_sb[st][:, n0:n0 + nw], in_=ps[:, :nw])

        # mask tiles loaded after projIn closes (reuses freed SBUF)
        mask_sb = []
        for st in range(NST):
            t = persist.tile([128, ROWS], dt.bfloat16, tag=f"mask{st}", name=f"mask{st}")
            nc.sync.dma_start(out=t, in_=maskT[st * 128:(st + 1) * 128, :])
            mask_sb.append(t)

        # ---------------- phase B: attention ----------------
        # Head-PAIR processing with explicit tile_position packing:
        #  - scores: the two heads' K=64 matmuls row-packed at (0,0)/(64,0)
        #  - ctx:    the two heads' M=64 matmuls col-packed at (0,0)/(0,64)
        #            into ONE [128, ROWS] psum tile (head h rows 0:64, h' 64:128)
        #  - Z:      4x M=1 ones-matmuls col-packed at (0,32j)
        # Emission is software-pipelined (ctx for step k after scores for k+1)
        # so the in-order PE stream doesn't stall on the exp->mask chain.
        ones_col = persist.tile([128, 1], dt.bfloat16, tag="ones_col", name="ones_col")
        nc.vector.memset(ones_col, 1.0)
        ctxn = [persist.tile([128, ROWS], dt.bfloat16, tag=f"ctxn{i}", name=f"ctxn{i}")
                for i in range(HP)]
        # zps row 32j holds Z for (sq-half ni, head h2), j = 2*ni + h2
        ZJ = [(0, 0), (1, 0), (0, 512), (1, 512)]  # (h2, n0) per zps row 32j
        with tc.tile_pool(name="attn", bufs=6) as attn, \
             tc.tile_pool(name="z97", bufs=2) as z97p, \
             tc.tile_pool(name="z2", bufs=2) as z2p, \
             tc.tile_pool(name="rzbc", bufs=2) as rzbcp, \
             tc.tile_pool(name="psS", bufs=2, space="PSUM") as psS, \
             tc.tile_pool(name="psC", bufs=1, space="PSUM") as psC, \
             tc.tile_pool(name="psZ", bufs=1, space="PSUM") as psZ, \
             tc.tile_pool(name="psK", bufs=1, space="PSUM") as psK:

            pending = None
            ctxp_cur = None
            zps_cur = None

            def emit_ctx(hp, st, pp, ctxp, zps):
                # pp[ni] holds [P_h0 sq-half ni | P_h1 sq-half ni] on the free axis
                for ni in range(2):
                    n0 = 512 * ni
                    # alternate member order between pairs so each pair's lead
                    # LDWEIGHTS targets the column group the previous pair just
                    # freed (lets both pairs fully pack)
                    for h2 in ((0, 1) if ni == 0 else (1, 0)):
                        nc.tensor.matmul(
                            ctxp[64 * h2:64 * h2 + 64, n0:n0 + 512],
                            v_sb[st][:, (2 * hp + h2) * HD:(2 * hp + h2 + 1) * HD],
                            pp[ni][:, 512 * h2:512 * h2 + 512],
                            start=(st == 0), stop=(st == NST - 1),
                            tile_position=(0, 64 * h2))
                for j, (h2, n0) in enumerate(ZJ):
                    ni = n0 // 512
                    nc.tensor.matmul(
                        zps[32 * j:32 * j + 1, 0:512],
                        ones_col,
                        pp[ni][:, 512 * h2:512 * h2 + 512],
                        start=(st == 0), stop=(st == NST - 1),
                        tile_position=(0, 32 * j))

            def drain_pair(hp, ctxp, zps):
                nc.vector.tensor_copy(out=ctx_sb[hp], in_=ctxp)
                z97 = z97p.tile([97, 512], dt.float32, tag="z97", name="z97")
                nc.vector.tensor_copy(out=z97, in_=zps[0:97, 0:512])
                z2 = z2p.tile([2, ROWS], dt.float32, tag="z2", name="z2")
                for j, (h2, n0) in enumerate(ZJ):
                    nc.sync.dma_start(out=z2[h2:h2 + 1, n0:n0 + 512],
                                      in_=z97[32 * j:32 * j + 1, :])
                rz2 = z2p.tile([2, ROWS], dt.float32, tag="rz2", name="rz2")
                nc.vector.reciprocal_approx_fast(out=rz2, in_=z2)
                rz2h = z2p.tile([2, ROWS], dt.bfloat16, tag="rz2h", name="rz2h")
                nc.vector.tensor_copy(out=rz2h, in_=rz2)
                nc.sync.dma_start(out=rz_dram[2 * hp:2 * hp + 2, :], in_=rz2h)
                bc = rzbcp.tile([128, ROWS], dt.bfloat16, tag="rzbc", name="rzbc")
                srcap = rz_dram[2 * hp:2 * hp + 2, :]
                bcast = bass.AP(tensor=srcap.tensor, offset=srcap.offset,
                                ap=[srcap.ap[0], [0, HD], srcap.ap[1]])
                nc.sync.dma_start(out=bc, in_=bcast)
                nc.vector.tensor_mul(out=ctxn[hp], in0=ctx_sb[hp], in1=bc)

            # interleaved KT[hp+1] rounds at steps 2/6/10/14 of pair hp
            KT_AT = {2: 0, 6: 1, 10: 2, 14: 3}
            for hp in range(HP):
                for st in range(NST):
                    if hp < HP - 1 and st in KT_AT:
                        kt_round(hp + 1, KT_AT[st] * 512, psK)
                    c0 = st * 128
                    if st == 0:
                        ctxp_cur = psC.tile([128, ROWS], dt.float32,
                                            tag="ctxp", name="ctxp")
                        zps_cur = psZ.tile([128, 512], dt.float32,
                                           tag="zps", name="zps")
                    # sps tile ni = [scores_h0 sq-half ni | scores_h1 sq-half ni]:
                    # both row-packed pair members land in ONE tile, so their
                    # slot dependencies resolve together and pairs never break.
                    pp = []
                    for ni in range(2):
                        n0 = 512 * ni
                        sps = psS.tile([128, ROWS], dt.float32, tag="sps", name="sps")
                        for h2 in ((0, 1) if ni == 0 else (1, 0)):
                            nc.tensor.matmul(
                                sps[:, 512 * h2:512 * h2 + 512],
                                kt_sb[hp][HD * h2:HD * h2 + HD, c0:c0 + 128],
                                qt_sb[hp][HD * h2:HD * h2 + HD, n0:n0 + 512],
                                start=True, stop=True,
                                tile_position=(64 * h2, 0))
                        p = attn.tile([128, ROWS], dt.bfloat16, tag="p", name="p")
                        nc.scalar.activation(out=p, in_=sps, func=EXP)
                        for h2 in range(2):
                            nc.vector.tensor_mul(
                                out=p[:, 512 * h2:512 * h2 + 512],
                                in0=p[:, 512 * h2:512 * h2 + 512],
                                in1=mask_sb[st][:, n0:n0 + 512])
                        pp.append(p)
                    if pending is not None:
                        php, pst, ppp, pctxp, pzps = pending
                        emit_ctx(php, pst, ppp, pctxp, pzps)
                        if pst == NST - 1:
                            drain_pair(php, pctxp, pzps)
                    pending = (hp, st, pp, ctxp_cur, zps_cur)
            php, pst, ppp, pctxp, pzps = pending
            emit_ctx(php, pst, ppp, pctxp, pzps)
            drain_pair(php, pctxp, pzps)

        projB_cm.__exit__(None, None, None)

        # ---------------- phase C: output projection ----------------
        with tc.tile_pool(name="outsb", bufs=2) as outsbp, \
             tc.tile_pool(name="psO", bufs=4, space="PSUM") as psO:
            for of in range(HP):
                o = outsbp.tile([128, ROWS], dt.float32, tag="outsb", name="outsb")
                for n0 in range(0, ROWS, 512):
                    ps = psO.tile([128, 512], dt.float32, tag="psO", name="psO")
                    for k in range(HP):
                        nc.tensor.matmul(
                            ps, wo_sb[k][:, of * 128:(of + 1) * 128],
                            ctxn[k][:, n0:n0 + 512],
                            start=(k == 0), stop=(k == HP - 1))
                    nc.vector.tensor_scalar_add(
                        out=o[:, n0:n0 + 512], in0=ps,
                        scalar1=bias_sb[:, HP + of:HP + of + 1])
                nc.sync.dma_start(out=out[of * 128:(of + 1) * 128, :], in_=o)

        persist_cm.__exit__(None, None, None)

    nc.compile()
    return nc


def _get_nc():
    global _CACHED_NC
    if _CACHED_NC is None:
        _CACHED_NC = _build_nc()
    return _CACHED_NC


def _prep_inputs(from_tensor, to_tensor, attention_mask,
                 Wq, bq, Wk, bk, Wv, bv, Wo, bo):
    f32 = np.float32
    from_tensor = np.asarray(from_tensor, f32)
    to_tensor = np.asarray(to_tensor, f32)
    attention_mask = np.asarray(attention_mask)

    Wq, bq = np.asarray(Wq, f32), np.asarray(bq, f32)
    Wk, bk = np.asarray(Wk, f32), np.asarray(bk, f32)
    Wv, bv = np.asarray(Wv, f32), np.asarray(bv, f32)
    Wo, bo = np.asarray(Wo, f32), np.asarray(bo, f32)
    wq_h = (Wq * SCALE).astype(BF16)
    wk_h = Wk.astype(BF16)
    wo_h = Wo.astype(BF16)
    wv_h = Wv.astype(BF16)

    # Exact bias folds:
    #  - bk: its score terms (q.bk, bq.bk) are constant along the softmax
    #    (St) axis and cancel in softmax -> dropped entirely
    #  - bv: softmax weights sum to 1, so ctx_norm = (P@V)/Z + bv exactly
    #    -> folded into the output bias: bo' = bv@Wo + bo
    #  - bq: kept, folded into the Q projection bias (scaled)
    bo_eff = bv @ Wo + bo
    biases = np.zeros((128, 2 * HP), f32)
    biases[:, 0:HP] = (bq * SCALE).reshape(HP, 128).T
    biases[:, HP:2 * HP] = bo_eff.reshape(HP, 128).T

    xtT_all = [np.ascontiguousarray(to_tensor[b].T).astype(BF16) for b in range(B)]

    in_maps = []
    for c in range(NCORES):
        b, half = c // 2, c % 2
        r0 = half * ROWS
        xfT = np.ascontiguousarray(from_tensor[b, r0:r0 + ROWS, :].T).astype(BF16)
        maskT = np.ascontiguousarray(
            attention_mask[b, r0:r0 + ROWS, :].T).astype(BF16)
        in_maps.append({
            "xfT": xfT, "xtT": xtT_all[b], "maskT": maskT,
            "wq": wq_h, "wk": wk_h, "wv": wv_h, "wo": wo_h,
            "biases": biases,
        })
    return in_maps


def _assemble(results):
    out = np.empty((B, SF, DIM), np.float32)
    for c, r in enumerate(results):
        b, half = c // 2, c % 2
        r0 = half * ROWS
        out[b, r0:r0 + ROWS, :] = np.asarray(r["out"], np.float32).T
    return out


def _run(in_maps, trace=False):
    from concourse.bass_utils import run_bass_kernel_spmd
    nc = _get_nc()
    return run_bass_kernel_spmd(nc, in_maps, core_ids=list(range(NCORES)),
                                trace=trace)


def kernel(**inputs):
    in_maps = _prep_inputs(**inputs)
    res = _run(in_maps, trace=False)
    return _assemble(res.results)


def kernel_profiled(**inputs):
    """Returns (output, exec_time_ns, trace_path)."""
    in_maps = _prep_inputs(**inputs)
    res = _run(in_maps, trace=True)
    trace_path = None
    if res.instructions_and_trace is not None:
        trace_path = res.instructions_and_trace[1]
    return _assemble(res.results), res.exec_time_ns, trace_path
